# revision 12
# baseline (speedup 1.0000x reference)
"""Adaptive Gaussian Bilateral Filter (nn_AGBF) — Trainium2 Bass kernel, 8 NeuronCores.

Problem: a tiny two-layer attention net predicts per-patch (16x16) sigmas
(sx, sy, sr); a k x k bilateral filter (k data-dependent, k=11 for the given
inputs) is applied to a [2,1,512,512] image with reflect padding.

Sharding: 8 cores x 128 image rows (4 cores per batch element). Each core:
  - runs the sigma net for its batch redundantly (attention over all 1024
    patches; queries restricted to its own 256 patches via host-side column
    permutation of the patch matrix, so the SPMD program is core-independent),
  - computes the bilateral filter for its 256 blocks in a blocks-on-partitions
    layout [128 blocks, 256 pixels] (2 chunks): sigma is constant per block,
    so the spatial column term folds into the per-partition bias of the ACT
    exp, the range scale into the per-partition ACT scale, and the spatial row
    term into diag(gy[dy]) stationary matrices on the PE.
  - per tap column dx: DVE d=(Uc-Un) (2 half-instructions); d^2 in place on
    either DVE (tensor_tensor) or ACT (Square — same table set as Exp) to
    balance engine load; ACT w=exp(-a*sq-cx); DVE m=w*Un; PE diag-matmuls
    accumulate S0=sum w, S1=sum m into PSUM.  out = S1/(S0+1e-8).

v4 engine/latency notes: all inputs arrive in 4 packed DMAs (the Sync queue
issues one DMA per ~600ns, so many small DMAs serialize); the sigma-chain's
per-partition-scale multiplies run on the scalar engine (activation Copy with
an AP scale), which idles during that stretch; the bulk filter sub/square
instructions are deprioritized so the serial attention->sigma chain is never
queued behind a 1.6us DVE instruction.
"""

import math
import os
import sys
from functools import lru_cache

import numpy as np

sys.path.insert(0, "/opt/trn_rl_repo")

PS = 16
HD = 8
B, C, H, W = 2, 1, 512, 512
NCORES = 8
RPC = (B * H) // NCORES          # 128 rows per core
BRC = RPC // PS                  # 8 block-rows per core
WB = W // PS                     # 32 block-cols
NBLK = BRC * WB                  # 256 blocks per core
PIX = PS * PS                    # 256 pixels per block
NP_ = (H // PS) * (W // PS)      # 1024 patches per batch
DIN = C * PS * PS                # 256


# ----------------------------------------------------------------------------
# host-side sigma-net mirror (numpy) — used only to pick the data-dependent k
# ----------------------------------------------------------------------------

def _np_softmax(s):
    s = s - s.max(-1, keepdims=True)
    e = np.exp(s)
    return e / e.sum(-1, keepdims=True)


def _np_attn(q, k, v):
    s = np.einsum('bnd,bmd->bnm', q, k) * (HD ** -0.5)
    return np.einsum('bnm,bmd->bnd', _np_softmax(s), v)


def _np_patch_sigmas(x, Wq, bq, Wk, bk, Wv, bv, Wsq, bsq, Wsk, bsk, Wsv, bsv,
                     ln_g, ln_b, Wp, bp):
    b, c, h, w = x.shape
    hp, wp = h // PS, w // PS
    p = x.reshape(b, c, hp, PS, wp, PS).transpose(0, 2, 4, 1, 3, 5).reshape(b, hp * wp, c * PS * PS)
    feats = _np_attn(p @ Wq + bq, p @ Wk + bk, p @ Wv + bv)
    sp = _np_attn(feats @ Wsq + bsq, feats @ Wsk + bsk, feats @ Wsv + bsv)
    mu = sp.mean(-1, keepdims=True)
    var = ((sp - mu) ** 2).mean(-1, keepdims=True)
    sp = (sp - mu) / np.sqrt(var + 1e-5) * ln_g + ln_b
    sp = sp @ Wp + bp
    sp = np.minimum(np.log1p(np.exp(sp)), 6.0) + 1e-6
    return sp  # [B, NP, 3]


def _pick_k(inputs):
    sp = _np_patch_sigmas(**{k: np.asarray(v) for k, v in inputs.items()})
    m = float(max(sp[..., 0].max(), sp[..., 1].max()))
    k = int(2 * math.ceil(m) + 1)
    if k % 2 == 0:
        k += 1
    return k


# ----------------------------------------------------------------------------
# device kernel builder
# ----------------------------------------------------------------------------

def _split_multi_waits(nc, mybir):
    """This container's walrus accepts only ONE sync wait per instruction;
    hoist extra waits onto inserted wait-only NoOps on the same engine."""
    for f in nc.m.functions:
        for blk in f.blocks:
            new_insts = []
            for inst in blk.instructions:
                si = inst.sync_info
                if si is not None and si.on_wait and len(si.on_wait) > 1:
                    extra, keep = si.on_wait[:-1], si.on_wait[-1:]
                    for j, wt in enumerate(extra):
                        nop = mybir.InstNoOp(
                            name=f"{inst.name}-ws{j}", ins=[], outs=[],
                            sync_info=mybir.SyncInfo(on_wait=[wt], on_update=[]))
                        nop.engine = inst.engine
                        new_insts.append(nop)
                    si.on_wait[:] = keep
                new_insts.append(inst)
            blk.instructions[:] = new_insts


def _register_sqdiff():
    """Register a fused (a-b)^2 custom DVE op (one pass instead of sub+mult).
    Follows the documented extension recipe in concourse/dve_ops.py."""
    from concourse import dve_ops
    from concourse.dve_spec import Spec, Src0, Src1, sq
    if 'SQDIFF_ANT' in dve_ops._SUB_OPCODE_FOR_NAME:
        return next(o for o in dve_ops.OPS if o.name == 'SQDIFF_ANT')
    op = dve_ops.DveOp(
        'SQDIFF_ANT',
        Spec(body=sq(Src0 - Src1),
             reference=lambda in0, in1, s0, s1, imm2: ((in0 - in1) ** 2).astype(np.float32)),
        subdim=False,
        uops_sha={'v3': 'eed49934a849c087', 'v4': 'cee42896e85173b8'},
    )
    dve_ops.OPS.append(op)
    dve_ops.CUSTOM_DVE_SPECS[op.name] = op.spec
    dve_ops._SUB_OPCODE_FOR_NAME[op.name] = (
        dve_ops._CUSTOM_DVE_ROW_BASE + len(dve_ops.OPS) - 1)
    return op


@lru_cache(maxsize=4)
def _build(k, nact=9, depri=1):
    import contextlib

    import concourse.bass as bass
    import concourse.tile as tile
    from concourse import mybir
    from concourse.mybir import AluOpType as Alu

    sqdiff_op = _register_sqdiff() if nact < 0 else None

    F32 = mybir.dt.float32
    BF16 = mybir.dt.bfloat16
    Act = mybir.ActivationFunctionType

    p = k // 2
    WIN = PS + 2 * p            # window side (even, since PS even and 2p even)
    WELEM = WIN * WIN
    CTR = p * WIN + p           # center offset in window

    nc = bass.Bass()
    BF = mybir.dt.bfloat16

    # packed-input column offsets (bf16 bank)
    O_XH0, O_XH1 = 0, WELEM
    O_W = 2 * WELEM                       # wq0|wq1|wk0|wk1|wv0|wv1  (6*HD cols)
    O_IDB = O_W + 6 * HD
    O_PT0 = O_IDB + 128
    O_PT1 = O_PT0 + NP_
    TOT16 = O_PT1 + NP_
    CUT16 = O_PT0                          # first DMA covers [0, CUT16)
    CB = 4 * HD + k                        # fp32 bank: idf | bv|bsv|lng|lnb|dsq
    TOTF = 128 + CB

    big_in = nc.declare_dram_parameter('big16', [128, TOT16], BF, isOutput=False)
    cf_in = nc.declare_dram_parameter('cf32', [128, TOTF], F32, isOutput=False)
    sb_in = nc.declare_dram_parameter('smallb', [HD, 3 * HD + 3], BF, isOutput=False)
    bvec_in = nc.declare_dram_parameter('bvec', [HD, 5], F32, isOutput=False)
    out_ext = nc.declare_dram_parameter('out', [NBLK, PIX], F32, isOutput=True)

    def view(t, extra_off, dims):
        return bass.AP(tensor=t.tensor, offset=t.offset + extra_off,
                       ap=[list(t.ap[0])] + [list(d) for d in dims])

    with tile.TileContext(nc) as tc:
        depri_ctx = (lambda: tc.high_priority(offset=-500000)) if depri else contextlib.nullcontext

        with tc.tile_pool(name='persist', bufs=1) as pp, \
             tc.tile_pool(name='work', bufs=2) as wkp, \
             tc.tile_pool(name='et', bufs=2) as etp:

            # ---- packed constant / input loads -------------------------
            big = pp.tile([128, TOT16], BF16, tag='big')
            nc.sync.dma_start(out=big[:, 0:CUT16], in_=big_in[:, 0:CUT16])
            nc.sync.dma_start(out=big[:, CUT16:TOT16], in_=big_in[:, CUT16:TOT16])
            cf = pp.tile([128, TOTF], F32, tag='cf')
            nc.sync.dma_start(out=cf[:], in_=cf_in[:])
            sb16 = pp.tile([HD, 3 * HD + 3], BF16, tag='sb16')
            nc.sync.dma_start(out=sb16[:], in_=sb_in[:])
            bvec_sb = pp.tile([HD, 5], F32, tag='bvec')
            nc.sync.dma_start(out=bvec_sb[:], in_=bvec_in[:])

            xb = [big[:, O_XH0:O_XH0 + WELEM], big[:, O_XH1:O_XH1 + WELEM]]
            wq_sb = [big[:, O_W + HD * i:O_W + HD * i + HD] for i in range(2)]
            wk_sb = [big[:, O_W + HD * (2 + i):O_W + HD * (2 + i) + HD] for i in range(2)]
            wv_sb = [big[:, O_W + HD * (4 + i):O_W + HD * (4 + i) + HD] for i in range(2)]
            idb = big[:, O_IDB:O_IDB + 128]
            pt_sb = [big[:, O_PT0:O_PT0 + NP_], big[:, O_PT1:O_PT1 + NP_]]
            idf = cf[:, 0:128]
            bv_b = cf[:, 128:128 + HD]
            bsv_b = cf[:, 128 + HD:128 + 2 * HD]
            lng_b = cf[:, 128 + 2 * HD:128 + 3 * HD]
            lnb_b = cf[:, 128 + 3 * HD:128 + 4 * HD]
            dsq_b = cf[:, 128 + 4 * HD:128 + 4 * HD + k]
            wsq_sb = sb16[:, 0:HD]
            wsk_sb = sb16[:, HD:2 * HD]
            wsv_sb = sb16[:, 2 * HD:3 * HD]
            wp_sb = sb16[:, 3 * HD:3 * HD + 3]
            bq_c, bk_c = bvec_sb[:, 0:1], bvec_sb[:, 1:2]
            bsq_c, bsk_c = bvec_sb[:, 2:3], bvec_sb[:, 3:4]
            bp_c = bvec_sb[0:3, 4:5]

            eps1 = pp.tile([128, 1], F32, tag='eps1')
            nc.vector.memset(eps1[:], 1e-5)
            one3 = pp.tile([3, 1], F32, tag='one3')
            nc.vector.memset(one3[:], 1.0)

            xbo = [pp.tile([128, WELEM], BF16, tag=f'xbo{i}', name=f'xbo{i}') for i in range(2)]
            for i in range(2):
                nc.scalar.copy(out=xbo[i][:, 0:WELEM - 1], in_=xb[i][:, 1:WELEM])

            NB2 = NP_ // 512  # psum banks per 1024-wide row
            HD1 = HD + 1      # feats rows + ones row for the denominator

            # ---- attention 1 (all 1024 patches of this core's batch) ----
            qT_sb = pp.tile([HD, NP_], BF16, tag='qT')
            kT_sb = pp.tile([HD, NP_], BF16, tag='kT')
            v_sb = pp.tile([128, HD1 * 8], BF16, tag='v')
            with tc.tile_pool(name='psA', bufs=2, space='PSUM') as psA:
                for (w_sb, b_c, dst) in ((wq_sb, bq_c, qT_sb), (wk_sb, bk_c, kT_sb)):
                    qk_ps = psA.tile([HD, NP_], F32, tag='big')
                    for bank in range(NB2):
                        for fh in range(2):
                            nc.tensor.matmul(qk_ps[:, 512 * bank:512 * bank + 512],
                                             w_sb[fh], pt_sb[fh][:, 512 * bank:512 * bank + 512],
                                             start=(fh == 0), stop=(fh == 1))
                    nc.scalar.add(out=dst[:], in_=qk_ps[:], add=b_c)
                for chn in range(8):
                    v_ps = psA.tile([128, HD], F32, tag='big', name=f'v_ps{chn}')
                    for fh in range(2):
                        nc.tensor.matmul(v_ps[:], pt_sb[fh][:, 128 * chn:128 * chn + 128],
                                         wv_sb[fh], start=(fh == 0), stop=(fh == 1))
                    nc.vector.tensor_add(out=v_sb[:, HD1 * chn:HD1 * chn + HD], in0=v_ps[:], in1=bv_b)
                    nc.vector.memset(v_sb[:, HD1 * chn + HD:HD1 * chn + HD1], 1.0)

                fT_ps = psA.tile([HD1, NP_], F32, tag='fT', bufs=1)
                eT_list = [None] * 8

                def ft_accum(kc):
                    for bank in range(NB2):
                        nc.tensor.matmul(fT_ps[:, 512 * bank:512 * bank + 512],
                                         v_sb[:, HD1 * kc:HD1 * kc + HD1],
                                         eT_list[kc][:, 512 * bank:512 * bank + 512],
                                         start=(kc == 0), stop=(kc == 7), skip_group_check=True)

                for kc in range(8):
                    sT_ps = psA.tile([128, NP_], F32, tag='big', name=f'sT_ps{kc}')
                    for bank in range(NB2):
                        nc.tensor.matmul(sT_ps[:, 512 * bank:512 * bank + 512],
                                         kT_sb[:, 128 * kc:128 * kc + 128],
                                         qT_sb[:, 512 * bank:512 * bank + 512],
                                         start=True, stop=True)
                    eT = etp.tile([128, NP_], BF16, tag='eT', name=f'eT{kc}', bufs=3)
                    nc.scalar.activation(out=eT[:], in_=sT_ps[:],
                                         func=Act.Exp, scale=HD ** -0.5)
                    eT_list[kc] = eT
                    if kc >= 1:
                        ft_accum(kc - 1)
                ft_accum(7)
                fT_sb = pp.tile([HD1, NP_], F32, tag='fTs')
                nc.scalar.copy(out=fT_sb[:], in_=fT_ps[:])

            # normalize feats: transpose [HD1, 128] chunks -> [128, HD1]; den = col HD
            fnT_sb = pp.tile([HD, NP_], BF16, tag='fnT')
            with tc.tile_pool(name='psB', bufs=1, space='PSUM') as psB:
                f_all = psB.tile([128, 8 * HD1], F32, tag='fnTp', bufs=1)
                for qc in range(8):
                    nc.tensor.transpose(f_all[:, HD1 * qc:HD1 * qc + HD1],
                                        fT_sb[:, 128 * qc:128 * qc + 128], idf[0:HD1, 0:HD1])
                f_sb = wkp.tile([128, 8 * HD1], F32, tag='fsb')
                nc.scalar.copy(out=f_sb[:], in_=f_all[:])
                dn_r = wkp.tile([128, 8], F32, tag='dnr8')
                nc.vector.reciprocal(out=dn_r[:], in_=view(f_sb, HD, [[HD1, 8], [1, 1]]))
                fn_all = wkp.tile([128, 8 * HD], F32, tag='fnall')
                for qc in range(8):
                    nc.scalar.activation(out=fn_all[:, HD * qc:HD * qc + HD],
                                         in_=f_sb[:, HD1 * qc:HD1 * qc + HD],
                                         func=Act.Copy, scale=dn_r[:, qc:qc + 1])
                fnT_ps = psB.tile([HD, NP_], F32, tag='fnTp', bufs=1)
                for qc in range(8):
                    nc.tensor.transpose(fnT_ps[:, 128 * qc:128 * qc + 128],
                                        fn_all[:, HD * qc:HD * qc + HD], idf[:])
                nc.scalar.copy(out=fnT_sb[:], in_=fnT_ps[:])

                # ---- attention 2 (queries = this core's first 256 patches) --
                q2T_sb = pp.tile([HD, NBLK], BF16, tag='q2T')
                k2T_sb = pp.tile([HD, NP_], BF16, tag='k2T')
                v2_sb = pp.tile([128, HD1 * 8], BF16, tag='v2')
                q2_ps = psB.tile([HD, NBLK], F32, tag='tp4')
                nc.tensor.matmul(q2_ps[:], wsq_sb, fnT_sb[:, 0:NBLK], start=True, stop=True)
                nc.scalar.add(out=q2T_sb[:], in_=q2_ps[:], add=bsq_c)
                for bank in range(NB2):
                    k2_ps = psB.tile([HD, 512], F32, tag='tp4')
                    nc.tensor.matmul(k2_ps[:], wsk_sb, fnT_sb[:, 512 * bank:512 * bank + 512],
                                     start=True, stop=True)
                    nc.scalar.add(out=k2T_sb[:, 512 * bank:512 * bank + 512], in_=k2_ps[:], add=bsk_c)
                for chn in range(8):
                    v2_ps = psB.tile([128, HD], F32, tag='tp')
                    nc.tensor.matmul(v2_ps[:], fnT_sb[:, 128 * chn:128 * chn + 128], wsv_sb,
                                     start=True, stop=True)
                    nc.vector.tensor_add(out=v2_sb[:, HD1 * chn:HD1 * chn + HD], in0=v2_ps[:], in1=bsv_b)
                    nc.vector.memset(v2_sb[:, HD1 * chn + HD:HD1 * chn + HD1], 1.0)

                spT_ps = psB.tile([HD1, NBLK], F32, tag='spT')
                for wave in range(2):
                    s2_ps = psB.tile([128, 4 * NBLK], F32, tag='s2', name=f's2w{wave}', bufs=1)
                    for j in range(4):
                        kc = 4 * wave + j
                        nc.tensor.matmul(s2_ps[:, NBLK * j:NBLK * j + NBLK],
                                         k2T_sb[:, 128 * kc:128 * kc + 128], q2T_sb[:],
                                         start=True, stop=True)
                    e2w = etp.tile([128, 4 * NBLK], BF16, tag='e2', name=f'e2w{wave}', bufs=2)
                    nc.scalar.activation(out=e2w[:], in_=s2_ps[:, 0:4 * NBLK],
                                         func=Act.Exp, scale=HD ** -0.5)
                    for j in range(4):
                        kc = 4 * wave + j
                        nc.tensor.matmul(spT_ps[:], v2_sb[:, HD1 * kc:HD1 * kc + HD1],
                                         e2w[:, NBLK * j:NBLK * j + NBLK],
                                         start=(kc == 0), stop=(kc == 7), skip_group_check=True)
                spT_sb = pp.tile([HD1, NBLK], F32, tag='spTs')
                nc.scalar.copy(out=spT_sb[:], in_=spT_ps[:])

                # ---- per-q-chunk: normalize, LN, project, softplus ------
                sig_sb = pp.tile([3, NBLK], F32, tag='sig')
                xnT_sb = pp.tile([HD, NBLK], BF16, tag='xnT')
                for qc in range(2):
                    sl = slice(128 * qc, 128 * qc + 128)
                    sp_ps = psB.tile([128, HD1], F32, tag='tp')
                    nc.tensor.transpose(sp_ps[:], spT_sb[:, sl], idf[0:HD1, 0:HD1])
                    d2_r = wkp.tile([128, 1], F32, tag='dnr')
                    nc.vector.reciprocal(out=d2_r[:], in_=sp_ps[:, HD:HD1])
                    spq = wkp.tile([128, HD], F32, tag='spq')
                    nc.scalar.activation(out=spq[:], in_=sp_ps[:, 0:HD],
                                         func=Act.Copy, scale=d2_r[:, 0:1])
                    # layernorm over HD
                    st = wkp.tile([128, nc.vector.BN_STATS_DIM], F32, tag='st')
                    nc.vector.bn_stats(out=st[:], in_=spq[:])
                    mv = wkp.tile([128, nc.vector.BN_AGGR_DIM], F32, tag='mv')
                    nc.vector.bn_aggr(out=mv[:], in_=st[:])
                    lnv = wkp.tile([128, 1], F32, tag='lnv')
                    nc.scalar.activation(out=lnv[:], in_=mv[:, 1:2], func=Act.Ln, bias=eps1[:, 0:1], scale=1.0)
                    rstd = wkp.tile([128, 1], F32, tag='rstd')
                    nc.scalar.activation(out=rstd[:], in_=lnv[:], func=Act.Exp, scale=-0.5)
                    xn = wkp.tile([128, HD], F32, tag='xn')
                    nc.vector.tensor_scalar(out=xn[:], in0=spq[:], scalar1=mv[:, 0:1], scalar2=rstd[:, 0:1],
                                            op0=Alu.subtract, op1=Alu.mult)
                    nc.vector.tensor_tensor(out=xn[:], in0=xn[:], in1=lng_b, op=Alu.mult)
                    nc.vector.tensor_tensor(out=xn[:], in0=xn[:], in1=lnb_b, op=Alu.add)
                    xnT_ps = psB.tile([HD, 128], F32, tag='tp3')
                    nc.tensor.transpose(xnT_ps[:], xn[:], idf[:])
                    nc.scalar.copy(out=xnT_sb[:, sl], in_=xnT_ps[:])
                lg_ps = psB.tile([3, NBLK], F32, tag='tp4')
                nc.tensor.matmul(lg_ps[:], wp_sb, xnT_sb[:], start=True, stop=True)
                lg_sb = pp.tile([3, NBLK], F32, tag='lg')
                nc.scalar.add(out=lg_sb[:], in_=lg_ps[:], add=bp_c)
                # bounded softplus: min(ln(1+exp(x)), 6) + 1e-6
                nc.scalar.activation(out=lg_sb[:], in_=lg_sb[:], func=Act.Exp, scale=1.0)
                nc.scalar.activation(out=lg_sb[:], in_=lg_sb[:], func=Act.Ln, bias=one3[:, 0:1], scale=1.0)
                nc.vector.tensor_scalar(out=sig_sb[:], in0=lg_sb[:], scalar1=6.0, scalar2=1e-6,
                                        op0=Alu.min, op1=Alu.add)

                # ---- per-chunk filter params -----------------------------
                negal, negcx, dgy = [], [], []
                for qc in range(2):
                    sl = slice(128 * qc, 128 * qc + 128)
                    sg_ps = psB.tile([128, 3], F32, tag='tp')
                    nc.tensor.transpose(sg_ps[:], sig_sb[:, sl], idf[0:3, 0:3])
                    sg = pp.tile([128, 3], F32, tag=f'sg{qc}', name=f'sg{qc}')
                    nc.scalar.copy(out=sg[:], in_=sg_ps[:])
                    n3 = pp.tile([128, 3], F32, tag=f'n3{qc}', name=f'n3{qc}')
                    nc.vector.reciprocal(out=n3[:], in_=sg[:])
                    nc.vector.tensor_tensor(out=n3[:], in0=n3[:], in1=n3[:], op=Alu.mult)
                    nc.vector.tensor_scalar_mul(out=n3[:], in0=n3[:], scalar1=-0.5)
                    negal.append(n3[:, 2:3])
                    ncx = pp.tile([128, k], F32, tag=f'ncx{qc}', name=f'ncx{qc}')
                    ncy = wkp.tile([128, k], F32, tag='ncy')
                    nc.scalar.activation(out=ncx[:], in_=dsq_b, func=Act.Copy, scale=n3[:, 0:1])
                    nc.scalar.activation(out=ncy[:], in_=dsq_b, func=Act.Copy, scale=n3[:, 1:2])
                    negcx.append(ncx)
                    # gy = exp(negcy); diag(gy[dy]) tiles for the PE accumulation
                    gyv = wkp.tile([128, k], F32, tag='gyv')
                    nc.scalar.activation(out=gyv[:], in_=ncy[:], func=Act.Exp, scale=1.0)
                    dg_list = []
                    with depri_ctx():
                        for dy in range(k):
                            dg = pp.tile([128, 128], BF16, tag=f'dgy{qc}_{dy}', name=f'dgy{qc}_{dy}')
                            nc.scalar.activation(out=dg[:], in_=idb, func=Act.Copy,
                                                 scale=gyv[:, dy:dy + 1])
                            dg_list.append(dg)
                    dgy.append(dg_list)

            # ---- bilateral filter hot loop (taps batched over dy) -------
            groups = [(qc, dx) for qc in range(2) for dx in range(k)]
            NG = len(groups)
            nact_eff = min(nact, NG)
            act_sq = set(range(NG - nact_eff, NG))   # tail groups square on ACT
            # fine-grained dy ranges: small bulk instructions cannot block the
            # serial sigma chain for long (head-of-line on the engine FIFO)
            QCUTS = [0, 3, 6, 9, k] if k >= 9 else [0, (k + 1) // 2, k]
            HCUTS = [0, (k + 1) // 2, k]

            with tc.tile_pool(name='psF', bufs=1, space='PSUM') as psF, \
                 tc.tile_pool(name='flt', bufs=3) as fp, \
                 tc.tile_pool(name='sqp', bufs=NG) as sqp:
                s01_ps = [psF.tile([128, 2 * PIX], F32, tag=f's01_{qc}', name=f's01_{qc}')
                          for qc in range(2)]

                sq_tiles = {}

                def emit_sub(gi):
                    qc, dx = groups[gi]
                    if CTR % 2 == 0:
                        uc_t, uc_off = xb[qc], CTR
                    else:
                        uc_t, uc_off = xbo[qc], CTR - 1
                    src, base = (xb[qc], dx) if dx % 2 == 0 else (xbo[qc], dx - 1)
                    sq = sqp.tile([128, k * PIX], BF16, tag='sq', name=f'sq{gi}')
                    for (lo, hi) in zip(QCUTS, QCUTS[1:]):
                        ucv = view(uc_t, uc_off, [[0, hi - lo], [WIN, PS], [1, PS]])
                        unv = view(src, base + lo * WIN, [[WIN, hi - lo], [WIN, PS], [1, PS]])
                        nc.vector.tensor_tensor(out=sq[:, lo * PIX:hi * PIX], in0=ucv, in1=unv,
                                                op=Alu.subtract)
                    unv_all = view(src, base, [[WIN, k], [WIN, PS], [1, PS]])
                    sq_tiles[gi] = (sq, unv_all)

                def emit_square(gi):
                    sq, _ = sq_tiles[gi]
                    for (lo, hi) in zip(HCUTS, HCUTS[1:]):
                        s = sq[:, lo * PIX:hi * PIX]
                        if gi in act_sq:
                            nc.scalar.activation(out=s, in_=s, func=Act.Square, scale=1.0)
                        else:
                            nc.vector.tensor_tensor(out=s, in0=s, in1=s, op=Alu.mult)

                # subs + squares for the ACT-squared (tail) groups first so the
                # scalar engine can chew them while it would otherwise idle;
                # everything here is deprioritized below the serial sigma chain.
                order = list(range(NG - nact_eff, NG)) + list(range(0, NG - nact_eff))
                with depri_ctx():
                    for gi in order:
                        emit_sub(gi)
                        emit_square(gi)

                def produce_wm(gi):
                    qc, dx = groups[gi]
                    sq, unv = sq_tiles.pop(gi)
                    wm = fp.tile([128, k * 2 * PIX], BF16, tag='wm', bufs=3, name=f'wm{gi}')
                    wv_ = view(wm, 0, [[2 * PIX, k], [1, PIX]])
                    nc.scalar.activation(out=wv_, in_=view(sq, 0, [[PIX, k], [1, PIX]]),
                                         func=Act.Exp, bias=negcx[qc][:, dx:dx + 1],
                                         scale=negal[qc][:, 0:1])
                    nc.vector.tensor_tensor(out=view(wm, PIX, [[2 * PIX, k], [1, PIX]]),
                                            in0=wv_, in1=unv, op=Alu.mult)
                    return wm

                def mm(gi, wm, dy):
                    qc = groups[gi][0]
                    nc.tensor.matmul(s01_ps[qc][:], dgy[qc][dy],
                                     wm[:, 2 * PIX * dy:2 * PIX * dy + 2 * PIX],
                                     start=(gi % k == 0 and dy == 0),
                                     stop=(gi % k == k - 1 and dy == k - 1),
                                     skip_group_check=True)

                for gi in range(NG):
                    wm0 = produce_wm(gi)
                    for dy in range(k):
                        mm(gi, wm0, dy)
                    if gi == k - 1 or gi == NG - 1:
                        # finalize this chunk: out = S1 / (S0 + 1e-8)
                        qc = groups[gi][0]
                        den = fp.tile([128, PIX], F32, tag='fden', name=f'fden{qc}')
                        nc.vector.tensor_scalar_add(out=den[:], in0=s01_ps[qc][:, 0:PIX], scalar1=1e-8)
                        nc.vector.reciprocal(out=den[:], in_=den[:])
                        res = fp.tile([128, PIX], F32, tag='fres', name=f'fres{qc}')
                        nc.vector.tensor_tensor(out=res[:], in0=s01_ps[qc][:, PIX:2 * PIX],
                                                in1=den[:], op=Alu.mult)
                        nc.sync.dma_start(out=out_ext[128 * qc:128 * qc + 128, :], in_=res[:])

    _split_multi_waits(nc, mybir)
    return nc


# ----------------------------------------------------------------------------
# host glue
# ----------------------------------------------------------------------------

def _prep_inputs(inputs, k):
    p = k // 2
    WIN = PS + 2 * p
    WELEM = WIN * WIN
    x = np.ascontiguousarray(np.asarray(inputs['x'], dtype=np.float32))
    xpad = np.pad(x[:, 0], ((0, 0), (p, p), (p, p)), mode='reflect')

    import ml_dtypes
    BFH = ml_dtypes.bfloat16

    wq = np.asarray(inputs['Wq'], np.float32)
    wk = np.asarray(inputs['Wk'], np.float32)
    wv = np.asarray(inputs['Wv'], np.float32)
    ident = np.eye(128, dtype=np.float32)

    TOT16 = 2 * WELEM + 6 * HD + 128 + 2 * NP_
    CB = 4 * HD + k

    sb = np.zeros((HD, 3 * HD + 3), np.float32)
    sb[:, 0:HD] = np.asarray(inputs['Wsq'], np.float32)
    sb[:, HD:2 * HD] = np.asarray(inputs['Wsk'], np.float32)
    sb[:, 2 * HD:3 * HD] = np.asarray(inputs['Wsv'], np.float32)
    sb[:, 3 * HD:3 * HD + 3] = np.asarray(inputs['Wp'], np.float32)

    bvec = np.zeros((HD, 5), np.float32)
    bvec[:, 0] = np.asarray(inputs['bq'], np.float32)
    bvec[:, 1] = np.asarray(inputs['bk'], np.float32)
    bvec[:, 2] = np.asarray(inputs['bsq'], np.float32)
    bvec[:, 3] = np.asarray(inputs['bsk'], np.float32)
    bvec[0:3, 4] = np.asarray(inputs['bp'], np.float32)

    offs = np.arange(-p, p + 1, dtype=np.float32)
    cf = np.zeros((128, 128 + CB), np.float32)
    cf[:, 0:128] = ident
    cf[:, 128:128 + HD] = np.asarray(inputs['bv'], np.float32).reshape(1, HD)
    cf[:, 128 + HD:128 + 2 * HD] = np.asarray(inputs['bsv'], np.float32).reshape(1, HD)
    cf[:, 128 + 2 * HD:128 + 3 * HD] = np.asarray(inputs['ln_g'], np.float32).reshape(1, HD)
    cf[:, 128 + 3 * HD:128 + 4 * HD] = np.asarray(inputs['ln_b'], np.float32).reshape(1, HD)
    cf[:, 128 + 4 * HD:128 + 4 * HD + k] = (offs ** 2).reshape(1, k)

    shared = {'cf32': cf, 'smallb': sb.astype(BFH), 'bvec': bvec}

    in_maps = []
    for c in range(NCORES):
        b, s = divmod(c, NCORES // B)
        slab = xpad[b, RPC * s: RPC * s + RPC + 2 * p, :]
        winv = np.lib.stride_tricks.sliding_window_view(slab, (WIN, WIN))[::PS, ::PS]
        xh = np.ascontiguousarray(winv.reshape(NBLK, WELEM), dtype=np.float32)
        xi = x[b, 0]
        pt = xi.reshape(H // PS, PS, W // PS, PS).transpose(1, 3, 0, 2).reshape(DIN, NP_)
        lo = NBLK * s
        perm = np.r_[lo:lo + NBLK, 0:lo, lo + NBLK:NP_]
        pt = np.ascontiguousarray(pt[:, perm], dtype=np.float32)
        big = np.zeros((128, TOT16), np.float32)
        big[:, 0:WELEM] = xh[0:128]
        big[:, WELEM:2 * WELEM] = xh[128:256]
        O_W = 2 * WELEM
        big[:, O_W + 0 * HD:O_W + 1 * HD] = wq[0:128]
        big[:, O_W + 1 * HD:O_W + 2 * HD] = wq[128:256]
        big[:, O_W + 2 * HD:O_W + 3 * HD] = wk[0:128]
        big[:, O_W + 3 * HD:O_W + 4 * HD] = wk[128:256]
        big[:, O_W + 4 * HD:O_W + 5 * HD] = wv[0:128]
        big[:, O_W + 5 * HD:O_W + 6 * HD] = wv[128:256]
        O_IDB = O_W + 6 * HD
        big[:, O_IDB:O_IDB + 128] = ident
        big[:, O_IDB + 128:O_IDB + 128 + NP_] = pt[0:128]
        big[:, O_IDB + 128 + NP_:O_IDB + 128 + 2 * NP_] = pt[128:256]
        m = dict(shared)
        m['big16'] = big.astype(BFH)
        in_maps.append(m)
    return in_maps


def _assemble(results):
    out = np.empty((B, C, H, W), dtype=np.float32)
    for c in range(NCORES):
        b, s = divmod(c, NCORES // B)
        r = results[c]['out']  # [NBLK, PIX]
        blk = r.reshape(BRC, WB, PS, PS).transpose(0, 2, 1, 3).reshape(RPC, W)
        out[b, 0, RPC * s: RPC * s + RPC, :] = blk
    return out


def kernel(**inputs):
    from concourse.bass_utils import run_bass_kernel_spmd
    k = _pick_k(inputs)
    nact = int(os.environ.get('AGBF_NACT', '4'))
    depri = int(os.environ.get('AGBF_DEPRI', '1'))
    nc = _build(k, nact, depri)
    in_maps = _prep_inputs(inputs, k)
    res = run_bass_kernel_spmd(nc, in_maps, core_ids=list(range(NCORES)),
                               trace=bool(int(os.environ.get('AGBF_TRACE', '0'))))
    out = _assemble(res.results)
    if os.environ.get('AGBF_TRACE', '0') != '0':
        kernel.last_exec_time_ns = res.exec_time_ns
    return out


# revision 14
# speedup vs baseline: 1.0048x; 1.0048x over previous
"""Adaptive Gaussian Bilateral Filter (nn_AGBF) — Trainium2 Bass kernel, 8 NeuronCores.

Problem: a tiny two-layer attention net predicts per-patch (16x16) sigmas
(sx, sy, sr); a k x k bilateral filter (k data-dependent, k=11 for the given
inputs) is applied to a [2,1,512,512] image with reflect padding.

Sharding: 8 cores x 128 image rows (4 cores per batch element). Each core:
  - runs the sigma net for its batch redundantly (attention over all 1024
    patches; queries restricted to its own 256 patches via host-side column
    permutation of the patch matrix, so the SPMD program is core-independent),
  - computes the bilateral filter for its 256 blocks in a blocks-on-partitions
    layout [128 blocks, 256 pixels] (2 chunks): sigma is constant per block,
    so the spatial column term folds into the per-partition bias of the ACT
    exp, the range scale into the per-partition ACT scale, and the spatial row
    term into diag(gy[dy]) stationary matrices on the PE.
  - per tap column dx: DVE d=(Uc-Un) (2 half-instructions); d^2 in place on
    either DVE (tensor_tensor) or ACT (Square — same table set as Exp) to
    balance engine load; ACT w=exp(-a*sq-cx); DVE m=w*Un; PE diag-matmuls
    accumulate S0=sum w, S1=sum m into PSUM.  out = S1/(S0+1e-8).

v4 engine/latency notes: all inputs arrive in 4 packed DMAs (the Sync queue
issues one DMA per ~600ns, so many small DMAs serialize); the sigma-chain's
per-partition-scale multiplies run on the scalar engine (activation Copy with
an AP scale), which idles during that stretch; the bulk filter sub/square
instructions are deprioritized so the serial attention->sigma chain is never
queued behind a 1.6us DVE instruction.
"""

import math
import os
import sys
from functools import lru_cache

import numpy as np

sys.path.insert(0, "/opt/trn_rl_repo")

PS = 16
HD = 8
B, C, H, W = 2, 1, 512, 512
NCORES = 8
RPC = (B * H) // NCORES          # 128 rows per core
BRC = RPC // PS                  # 8 block-rows per core
WB = W // PS                     # 32 block-cols
NBLK = BRC * WB                  # 256 blocks per core
PIX = PS * PS                    # 256 pixels per block
NP_ = (H // PS) * (W // PS)      # 1024 patches per batch
DIN = C * PS * PS                # 256


# ----------------------------------------------------------------------------
# host-side sigma-net mirror (numpy) — used only to pick the data-dependent k
# ----------------------------------------------------------------------------

def _np_softmax(s):
    s = s - s.max(-1, keepdims=True)
    e = np.exp(s)
    return e / e.sum(-1, keepdims=True)


def _np_attn(q, k, v):
    s = np.einsum('bnd,bmd->bnm', q, k) * (HD ** -0.5)
    return np.einsum('bnm,bmd->bnd', _np_softmax(s), v)


def _np_patch_sigmas(x, Wq, bq, Wk, bk, Wv, bv, Wsq, bsq, Wsk, bsk, Wsv, bsv,
                     ln_g, ln_b, Wp, bp):
    b, c, h, w = x.shape
    hp, wp = h // PS, w // PS
    p = x.reshape(b, c, hp, PS, wp, PS).transpose(0, 2, 4, 1, 3, 5).reshape(b, hp * wp, c * PS * PS)
    feats = _np_attn(p @ Wq + bq, p @ Wk + bk, p @ Wv + bv)
    sp = _np_attn(feats @ Wsq + bsq, feats @ Wsk + bsk, feats @ Wsv + bsv)
    mu = sp.mean(-1, keepdims=True)
    var = ((sp - mu) ** 2).mean(-1, keepdims=True)
    sp = (sp - mu) / np.sqrt(var + 1e-5) * ln_g + ln_b
    sp = sp @ Wp + bp
    sp = np.minimum(np.log1p(np.exp(sp)), 6.0) + 1e-6
    return sp  # [B, NP, 3]


def _pick_k(inputs):
    sp = _np_patch_sigmas(**{k: np.asarray(v) for k, v in inputs.items()})
    m = float(max(sp[..., 0].max(), sp[..., 1].max()))
    k = int(2 * math.ceil(m) + 1)
    if k % 2 == 0:
        k += 1
    return k


# ----------------------------------------------------------------------------
# device kernel builder
# ----------------------------------------------------------------------------

def _split_multi_waits(nc, mybir):
    """This container's walrus accepts only ONE sync wait per instruction;
    hoist extra waits onto inserted wait-only NoOps on the same engine."""
    for f in nc.m.functions:
        for blk in f.blocks:
            new_insts = []
            for inst in blk.instructions:
                si = inst.sync_info
                if si is not None and si.on_wait and len(si.on_wait) > 1:
                    extra, keep = si.on_wait[:-1], si.on_wait[-1:]
                    for j, wt in enumerate(extra):
                        nop = mybir.InstNoOp(
                            name=f"{inst.name}-ws{j}", ins=[], outs=[],
                            sync_info=mybir.SyncInfo(on_wait=[wt], on_update=[]))
                        nop.engine = inst.engine
                        new_insts.append(nop)
                    si.on_wait[:] = keep
                new_insts.append(inst)
            blk.instructions[:] = new_insts


def _register_sqdiff():
    """Register a fused (a-b)^2 custom DVE op (one pass instead of sub+mult).
    Follows the documented extension recipe in concourse/dve_ops.py."""
    from concourse import dve_ops
    from concourse.dve_spec import Spec, Src0, Src1, sq
    if 'SQDIFF_ANT' in dve_ops._SUB_OPCODE_FOR_NAME:
        return next(o for o in dve_ops.OPS if o.name == 'SQDIFF_ANT')
    op = dve_ops.DveOp(
        'SQDIFF_ANT',
        Spec(body=sq(Src0 - Src1),
             reference=lambda in0, in1, s0, s1, imm2: ((in0 - in1) ** 2).astype(np.float32)),
        subdim=False,
        uops_sha={'v3': 'eed49934a849c087', 'v4': 'cee42896e85173b8'},
    )
    dve_ops.OPS.append(op)
    dve_ops.CUSTOM_DVE_SPECS[op.name] = op.spec
    dve_ops._SUB_OPCODE_FOR_NAME[op.name] = (
        dve_ops._CUSTOM_DVE_ROW_BASE + len(dve_ops.OPS) - 1)
    return op


@lru_cache(maxsize=4)
def _build(k, nact=9, depri=1):
    import contextlib

    import concourse.bass as bass
    import concourse.tile as tile
    from concourse import mybir
    from concourse.mybir import AluOpType as Alu

    sqdiff_op = _register_sqdiff() if nact < 0 else None

    F32 = mybir.dt.float32
    BF16 = mybir.dt.bfloat16
    Act = mybir.ActivationFunctionType

    p = k // 2
    WIN = PS + 2 * p            # window side (even, since PS even and 2p even)
    WELEM = WIN * WIN
    CTR = p * WIN + p           # center offset in window

    nc = bass.Bass()
    BF = mybir.dt.bfloat16

    # packed-input column offsets (bf16 bank)
    O_XH0, O_XH1 = 0, WELEM
    O_W = 2 * WELEM                       # wq0|wq1|wk0|wk1|wv0|wv1  (6*HD cols)
    O_IDB = O_W + 6 * HD
    O_PT0 = O_IDB + 128
    O_PT1 = O_PT0 + NP_
    TOT16 = O_PT1 + NP_
    CUT16 = O_PT0                          # first DMA covers [0, CUT16)
    CB = 4 * HD + k                        # fp32 bank: idf | bv|bsv|lng|lnb|dsq
    TOTF = 128 + CB

    big_in = nc.declare_dram_parameter('big16', [128, TOT16], BF, isOutput=False)
    cf_in = nc.declare_dram_parameter('cf32', [128, TOTF], F32, isOutput=False)
    sb_in = nc.declare_dram_parameter('smallb', [HD, 3 * HD + 3], BF, isOutput=False)
    bvec_in = nc.declare_dram_parameter('bvec', [HD, 5], F32, isOutput=False)
    out_ext = nc.declare_dram_parameter('out', [NBLK, PIX], F32, isOutput=True)

    def view(t, extra_off, dims):
        return bass.AP(tensor=t.tensor, offset=t.offset + extra_off,
                       ap=[list(t.ap[0])] + [list(d) for d in dims])

    with tile.TileContext(nc) as tc:
        depri_ctx = (lambda: tc.high_priority(offset=-500000)) if depri else contextlib.nullcontext

        with tc.tile_pool(name='persist', bufs=1) as pp, \
             tc.tile_pool(name='work', bufs=2) as wkp, \
             tc.tile_pool(name='et', bufs=2) as etp:

            # ---- packed constant / input loads -------------------------
            big = pp.tile([128, TOT16], BF16, tag='big')
            nc.sync.dma_start(out=big[:, 0:CUT16], in_=big_in[:, 0:CUT16])
            nc.sync.dma_start(out=big[:, CUT16:TOT16], in_=big_in[:, CUT16:TOT16])
            cf = pp.tile([128, TOTF], F32, tag='cf')
            nc.sync.dma_start(out=cf[:], in_=cf_in[:])
            sb16 = pp.tile([HD, 3 * HD + 3], BF16, tag='sb16')
            nc.sync.dma_start(out=sb16[:], in_=sb_in[:])
            bvec_sb = pp.tile([HD, 5], F32, tag='bvec')
            nc.sync.dma_start(out=bvec_sb[:], in_=bvec_in[:])

            xb = [big[:, O_XH0:O_XH0 + WELEM], big[:, O_XH1:O_XH1 + WELEM]]
            wq_sb = [big[:, O_W + HD * i:O_W + HD * i + HD] for i in range(2)]
            wk_sb = [big[:, O_W + HD * (2 + i):O_W + HD * (2 + i) + HD] for i in range(2)]
            wv_sb = [big[:, O_W + HD * (4 + i):O_W + HD * (4 + i) + HD] for i in range(2)]
            idb = big[:, O_IDB:O_IDB + 128]
            pt_sb = [big[:, O_PT0:O_PT0 + NP_], big[:, O_PT1:O_PT1 + NP_]]
            idf = cf[:, 0:128]
            bv_b = cf[:, 128:128 + HD]
            bsv_b = cf[:, 128 + HD:128 + 2 * HD]
            lng_b = cf[:, 128 + 2 * HD:128 + 3 * HD]
            lnb_b = cf[:, 128 + 3 * HD:128 + 4 * HD]
            dsq_b = cf[:, 128 + 4 * HD:128 + 4 * HD + k]
            wsq_sb = sb16[:, 0:HD]
            wsk_sb = sb16[:, HD:2 * HD]
            wsv_sb = sb16[:, 2 * HD:3 * HD]
            wp_sb = sb16[:, 3 * HD:3 * HD + 3]
            bq_c, bk_c = bvec_sb[:, 0:1], bvec_sb[:, 1:2]
            bsq_c, bsk_c = bvec_sb[:, 2:3], bvec_sb[:, 3:4]
            bp_c = bvec_sb[0:3, 4:5]

            eps1 = pp.tile([128, 1], F32, tag='eps1')
            nc.vector.memset(eps1[:], 1e-5)
            one3 = pp.tile([3, 1], F32, tag='one3')
            nc.vector.memset(one3[:], 1.0)

            xbo = [pp.tile([128, WELEM], BF16, tag=f'xbo{i}', name=f'xbo{i}') for i in range(2)]
            for i in range(2):
                nc.vector.tensor_copy(out=xbo[i][:, 0:WELEM - 1], in_=xb[i][:, 1:WELEM])

            NB2 = NP_ // 512  # psum banks per 1024-wide row
            HD1 = HD + 1      # feats rows + ones row for the denominator

            # ---- attention 1 (all 1024 patches of this core's batch) ----
            qT_sb = pp.tile([HD, NP_], BF16, tag='qT')
            kT_sb = pp.tile([HD, NP_], BF16, tag='kT')
            v_sb = pp.tile([128, HD1 * 8], BF16, tag='v')
            with tc.tile_pool(name='psA', bufs=2, space='PSUM') as psA:
                for (w_sb, b_c, dst) in ((wq_sb, bq_c, qT_sb), (wk_sb, bk_c, kT_sb)):
                    qk_ps = psA.tile([HD, NP_], F32, tag='big')
                    for bank in range(NB2):
                        for fh in range(2):
                            nc.tensor.matmul(qk_ps[:, 512 * bank:512 * bank + 512],
                                             w_sb[fh], pt_sb[fh][:, 512 * bank:512 * bank + 512],
                                             start=(fh == 0), stop=(fh == 1))
                    nc.scalar.add(out=dst[:], in_=qk_ps[:], add=b_c)
                for chn in range(8):
                    v_ps = psA.tile([128, HD], F32, tag='big', name=f'v_ps{chn}')
                    for fh in range(2):
                        nc.tensor.matmul(v_ps[:], pt_sb[fh][:, 128 * chn:128 * chn + 128],
                                         wv_sb[fh], start=(fh == 0), stop=(fh == 1))
                    nc.vector.tensor_add(out=v_sb[:, HD1 * chn:HD1 * chn + HD], in0=v_ps[:], in1=bv_b)
                    nc.vector.memset(v_sb[:, HD1 * chn + HD:HD1 * chn + HD1], 1.0)

                fT_ps = psA.tile([HD1, NP_], F32, tag='fT', bufs=1)
                eT_list = [None] * 8

                def ft_accum(kc):
                    for bank in range(NB2):
                        nc.tensor.matmul(fT_ps[:, 512 * bank:512 * bank + 512],
                                         v_sb[:, HD1 * kc:HD1 * kc + HD1],
                                         eT_list[kc][:, 512 * bank:512 * bank + 512],
                                         start=(kc == 0), stop=(kc == 7), skip_group_check=True)

                for kc in range(8):
                    sT_ps = psA.tile([128, NP_], F32, tag='big', name=f'sT_ps{kc}')
                    for bank in range(NB2):
                        nc.tensor.matmul(sT_ps[:, 512 * bank:512 * bank + 512],
                                         kT_sb[:, 128 * kc:128 * kc + 128],
                                         qT_sb[:, 512 * bank:512 * bank + 512],
                                         start=True, stop=True)
                    eT = etp.tile([128, NP_], BF16, tag='eT', name=f'eT{kc}', bufs=3)
                    nc.scalar.activation(out=eT[:], in_=sT_ps[:],
                                         func=Act.Exp, scale=HD ** -0.5)
                    eT_list[kc] = eT
                    if kc >= 1:
                        ft_accum(kc - 1)
                ft_accum(7)
                fT_sb = pp.tile([HD1, NP_], F32, tag='fTs')
                nc.scalar.copy(out=fT_sb[:], in_=fT_ps[:])

            # normalize feats: transpose [HD1, 128] chunks -> [128, HD1]; den = col HD
            fnT_sb = pp.tile([HD, NP_], BF16, tag='fnT')
            with tc.tile_pool(name='psB', bufs=1, space='PSUM') as psB:
                f_all = psB.tile([128, 8 * HD1], F32, tag='fnTp', bufs=1)
                for qc in range(8):
                    nc.tensor.transpose(f_all[:, HD1 * qc:HD1 * qc + HD1],
                                        fT_sb[:, 128 * qc:128 * qc + 128], idf[0:HD1, 0:HD1])
                f_sb = wkp.tile([128, 8 * HD1], F32, tag='fsb')
                nc.scalar.copy(out=f_sb[:], in_=f_all[:])
                dn_r = wkp.tile([128, 8], F32, tag='dnr8')
                nc.vector.reciprocal(out=dn_r[:], in_=view(f_sb, HD, [[HD1, 8], [1, 1]]))
                fn_all = wkp.tile([128, 8 * HD], F32, tag='fnall')
                for qc in range(8):
                    nc.scalar.activation(out=fn_all[:, HD * qc:HD * qc + HD],
                                         in_=f_sb[:, HD1 * qc:HD1 * qc + HD],
                                         func=Act.Copy, scale=dn_r[:, qc:qc + 1])
                fnT_ps = psB.tile([HD, NP_], F32, tag='fnTp', bufs=1)
                for qc in range(8):
                    nc.tensor.transpose(fnT_ps[:, 128 * qc:128 * qc + 128],
                                        fn_all[:, HD * qc:HD * qc + HD], idf[:])
                nc.scalar.copy(out=fnT_sb[:], in_=fnT_ps[:])

                # ---- attention 2 (queries = this core's first 256 patches) --
                q2T_sb = pp.tile([HD, NBLK], BF16, tag='q2T')
                k2T_sb = pp.tile([HD, NP_], BF16, tag='k2T')
                v2_sb = pp.tile([128, HD1 * 8], BF16, tag='v2')
                q2_ps = psB.tile([HD, NBLK], F32, tag='tp4')
                nc.tensor.matmul(q2_ps[:], wsq_sb, fnT_sb[:, 0:NBLK], start=True, stop=True)
                nc.scalar.add(out=q2T_sb[:], in_=q2_ps[:], add=bsq_c)
                for bank in range(NB2):
                    k2_ps = psB.tile([HD, 512], F32, tag='tp4')
                    nc.tensor.matmul(k2_ps[:], wsk_sb, fnT_sb[:, 512 * bank:512 * bank + 512],
                                     start=True, stop=True)
                    nc.scalar.add(out=k2T_sb[:, 512 * bank:512 * bank + 512], in_=k2_ps[:], add=bsk_c)
                for chn in range(8):
                    v2_ps = psB.tile([128, HD], F32, tag='tp')
                    nc.tensor.matmul(v2_ps[:], fnT_sb[:, 128 * chn:128 * chn + 128], wsv_sb,
                                     start=True, stop=True)
                    nc.vector.tensor_add(out=v2_sb[:, HD1 * chn:HD1 * chn + HD], in0=v2_ps[:], in1=bsv_b)
                    nc.vector.memset(v2_sb[:, HD1 * chn + HD:HD1 * chn + HD1], 1.0)

                spT_ps = psB.tile([HD1, NBLK], F32, tag='spT')
                for wave in range(2):
                    s2_ps = psB.tile([128, 4 * NBLK], F32, tag='s2', name=f's2w{wave}', bufs=1)
                    for j in range(4):
                        kc = 4 * wave + j
                        nc.tensor.matmul(s2_ps[:, NBLK * j:NBLK * j + NBLK],
                                         k2T_sb[:, 128 * kc:128 * kc + 128], q2T_sb[:],
                                         start=True, stop=True)
                    e2w = etp.tile([128, 4 * NBLK], BF16, tag='e2', name=f'e2w{wave}', bufs=2)
                    nc.scalar.activation(out=e2w[:], in_=s2_ps[:, 0:4 * NBLK],
                                         func=Act.Exp, scale=HD ** -0.5)
                    for j in range(4):
                        kc = 4 * wave + j
                        nc.tensor.matmul(spT_ps[:], v2_sb[:, HD1 * kc:HD1 * kc + HD1],
                                         e2w[:, NBLK * j:NBLK * j + NBLK],
                                         start=(kc == 0), stop=(kc == 7), skip_group_check=True)
                spT_sb = pp.tile([HD1, NBLK], F32, tag='spTs')
                nc.scalar.copy(out=spT_sb[:], in_=spT_ps[:])

                # ---- per-q-chunk: normalize, LN, project, softplus ------
                sig_sb = pp.tile([3, NBLK], F32, tag='sig')
                xnT_sb = pp.tile([HD, NBLK], BF16, tag='xnT')
                for qc in range(2):
                    sl = slice(128 * qc, 128 * qc + 128)
                    sp_ps = psB.tile([128, HD1], F32, tag='tp')
                    nc.tensor.transpose(sp_ps[:], spT_sb[:, sl], idf[0:HD1, 0:HD1])
                    d2_r = wkp.tile([128, 1], F32, tag='dnr')
                    nc.vector.reciprocal(out=d2_r[:], in_=sp_ps[:, HD:HD1])
                    spq = wkp.tile([128, HD], F32, tag='spq')
                    nc.scalar.activation(out=spq[:], in_=sp_ps[:, 0:HD],
                                         func=Act.Copy, scale=d2_r[:, 0:1])
                    # layernorm over HD
                    st = wkp.tile([128, nc.vector.BN_STATS_DIM], F32, tag='st')
                    nc.vector.bn_stats(out=st[:], in_=spq[:])
                    mv = wkp.tile([128, nc.vector.BN_AGGR_DIM], F32, tag='mv')
                    nc.vector.bn_aggr(out=mv[:], in_=st[:])
                    lnv = wkp.tile([128, 1], F32, tag='lnv')
                    nc.scalar.activation(out=lnv[:], in_=mv[:, 1:2], func=Act.Ln, bias=eps1[:, 0:1], scale=1.0)
                    rstd = wkp.tile([128, 1], F32, tag='rstd')
                    nc.scalar.activation(out=rstd[:], in_=lnv[:], func=Act.Exp, scale=-0.5)
                    xn = wkp.tile([128, HD], F32, tag='xn')
                    nc.vector.tensor_scalar(out=xn[:], in0=spq[:], scalar1=mv[:, 0:1], scalar2=rstd[:, 0:1],
                                            op0=Alu.subtract, op1=Alu.mult)
                    nc.vector.tensor_tensor(out=xn[:], in0=xn[:], in1=lng_b, op=Alu.mult)
                    nc.vector.tensor_tensor(out=xn[:], in0=xn[:], in1=lnb_b, op=Alu.add)
                    xnT_ps = psB.tile([HD, 128], F32, tag='tp3')
                    nc.tensor.transpose(xnT_ps[:], xn[:], idf[:])
                    nc.scalar.copy(out=xnT_sb[:, sl], in_=xnT_ps[:])
                lg_ps = psB.tile([3, NBLK], F32, tag='tp4')
                nc.tensor.matmul(lg_ps[:], wp_sb, xnT_sb[:], start=True, stop=True)
                lg_sb = pp.tile([3, NBLK], F32, tag='lg')
                nc.scalar.add(out=lg_sb[:], in_=lg_ps[:], add=bp_c)
                # bounded softplus: min(ln(1+exp(x)), 6) + 1e-6
                nc.scalar.activation(out=lg_sb[:], in_=lg_sb[:], func=Act.Exp, scale=1.0)
                nc.scalar.activation(out=lg_sb[:], in_=lg_sb[:], func=Act.Ln, bias=one3[:, 0:1], scale=1.0)
                nc.vector.tensor_scalar(out=sig_sb[:], in0=lg_sb[:], scalar1=6.0, scalar2=1e-6,
                                        op0=Alu.min, op1=Alu.add)

                # ---- per-chunk filter params -----------------------------
                negal, negcx, dgy = [], [], []
                for qc in range(2):
                    sl = slice(128 * qc, 128 * qc + 128)
                    sg_ps = psB.tile([128, 3], F32, tag='tp')
                    nc.tensor.transpose(sg_ps[:], sig_sb[:, sl], idf[0:3, 0:3])
                    sg = pp.tile([128, 3], F32, tag=f'sg{qc}', name=f'sg{qc}')
                    nc.scalar.copy(out=sg[:], in_=sg_ps[:])
                    n3 = pp.tile([128, 3], F32, tag=f'n3{qc}', name=f'n3{qc}')
                    nc.vector.reciprocal(out=n3[:], in_=sg[:])
                    nc.vector.tensor_tensor(out=n3[:], in0=n3[:], in1=n3[:], op=Alu.mult)
                    nc.vector.tensor_scalar_mul(out=n3[:], in0=n3[:], scalar1=-0.5)
                    negal.append(n3[:, 2:3])
                    ncx = pp.tile([128, k], F32, tag=f'ncx{qc}', name=f'ncx{qc}')
                    ncy = wkp.tile([128, k], F32, tag='ncy')
                    nc.scalar.activation(out=ncx[:], in_=dsq_b, func=Act.Copy, scale=n3[:, 0:1])
                    nc.scalar.activation(out=ncy[:], in_=dsq_b, func=Act.Copy, scale=n3[:, 1:2])
                    negcx.append(ncx)
                    # gy = exp(negcy); diag(gy[dy]) tiles for the PE accumulation
                    gyv = wkp.tile([128, k], F32, tag='gyv')
                    nc.scalar.activation(out=gyv[:], in_=ncy[:], func=Act.Exp, scale=1.0)
                    dg_list = []
                    for dy in range(k):
                        dg = pp.tile([128, 128], BF16, tag=f'dgy{qc}_{dy}', name=f'dgy{qc}_{dy}')
                        nc.scalar.activation(out=dg[:], in_=idb, func=Act.Copy,
                                             scale=gyv[:, dy:dy + 1])
                        dg_list.append(dg)
                    dgy.append(dg_list)

            # ---- bilateral filter hot loop (taps batched over dy) -------
            groups = [(qc, dx) for qc in range(2) for dx in range(k)]
            NG = len(groups)
            nact_eff = min(nact, NG)
            act_sq = set(range(NG - nact_eff, NG))   # tail groups square on ACT
            # fine-grained dy ranges: small bulk instructions cannot block the
            # serial sigma chain for long (head-of-line on the engine FIFO)
            QCUTS = [0, 3, 6, 9, k] if k >= 9 else [0, (k + 1) // 2, k]
            HCUTS = [0, (k + 1) // 2, k]

            with tc.tile_pool(name='psF', bufs=1, space='PSUM') as psF, \
                 tc.tile_pool(name='flt', bufs=3) as fp, \
                 tc.tile_pool(name='sqp', bufs=NG) as sqp:
                s01_ps = [psF.tile([128, 2 * PIX], F32, tag=f's01_{qc}', name=f's01_{qc}')
                          for qc in range(2)]

                sq_tiles = {}

                def emit_sub(gi):
                    qc, dx = groups[gi]
                    if CTR % 2 == 0:
                        uc_t, uc_off = xb[qc], CTR
                    else:
                        uc_t, uc_off = xbo[qc], CTR - 1
                    src, base = (xb[qc], dx) if dx % 2 == 0 else (xbo[qc], dx - 1)
                    sq = sqp.tile([128, k * PIX], BF16, tag='sq', name=f'sq{gi}')
                    for (lo, hi) in zip(QCUTS, QCUTS[1:]):
                        ucv = view(uc_t, uc_off, [[0, hi - lo], [WIN, PS], [1, PS]])
                        unv = view(src, base + lo * WIN, [[WIN, hi - lo], [WIN, PS], [1, PS]])
                        nc.vector.tensor_tensor(out=sq[:, lo * PIX:hi * PIX], in0=ucv, in1=unv,
                                                op=Alu.subtract)
                    unv_all = view(src, base, [[WIN, k], [WIN, PS], [1, PS]])
                    sq_tiles[gi] = (sq, unv_all)

                def emit_square(gi):
                    sq, _ = sq_tiles[gi]
                    for (lo, hi) in zip(HCUTS, HCUTS[1:]):
                        s = sq[:, lo * PIX:hi * PIX]
                        if gi in act_sq:
                            nc.scalar.activation(out=s, in_=s, func=Act.Square, scale=1.0)
                        else:
                            nc.vector.tensor_tensor(out=s, in0=s, in1=s, op=Alu.mult)

                # subs + squares for the ACT-squared (tail) groups first so the
                # scalar engine can chew them while it would otherwise idle;
                # everything here is deprioritized below the serial sigma chain.
                order = list(range(NG - nact_eff, NG)) + list(range(0, NG - nact_eff))
                with depri_ctx():
                    for gi in order:
                        emit_sub(gi)
                        emit_square(gi)

                def produce_wm(gi):
                    qc, dx = groups[gi]
                    sq, unv = sq_tiles.pop(gi)
                    wm = fp.tile([128, k * 2 * PIX], BF16, tag='wm', bufs=3, name=f'wm{gi}')
                    wv_ = view(wm, 0, [[2 * PIX, k], [1, PIX]])
                    nc.scalar.activation(out=wv_, in_=view(sq, 0, [[PIX, k], [1, PIX]]),
                                         func=Act.Exp, bias=negcx[qc][:, dx:dx + 1],
                                         scale=negal[qc][:, 0:1])
                    nc.vector.tensor_tensor(out=view(wm, PIX, [[2 * PIX, k], [1, PIX]]),
                                            in0=wv_, in1=unv, op=Alu.mult)
                    return wm

                def mm(gi, wm, dy):
                    qc = groups[gi][0]
                    nc.tensor.matmul(s01_ps[qc][:], dgy[qc][dy],
                                     wm[:, 2 * PIX * dy:2 * PIX * dy + 2 * PIX],
                                     start=(gi % k == 0 and dy == 0),
                                     stop=(gi % k == k - 1 and dy == k - 1),
                                     skip_group_check=True)

                for gi in range(NG):
                    wm0 = produce_wm(gi)
                    for dy in range(k):
                        mm(gi, wm0, dy)
                    if gi == k - 1 or gi == NG - 1:
                        # finalize this chunk: out = S1 / (S0 + 1e-8)
                        qc = groups[gi][0]
                        den = fp.tile([128, PIX], F32, tag='fden', name=f'fden{qc}')
                        nc.vector.tensor_scalar_add(out=den[:], in0=s01_ps[qc][:, 0:PIX], scalar1=1e-8)
                        nc.vector.reciprocal(out=den[:], in_=den[:])
                        res = fp.tile([128, PIX], F32, tag='fres', name=f'fres{qc}')
                        nc.vector.tensor_tensor(out=res[:], in0=s01_ps[qc][:, PIX:2 * PIX],
                                                in1=den[:], op=Alu.mult)
                        nc.sync.dma_start(out=out_ext[128 * qc:128 * qc + 128, :], in_=res[:])

    _split_multi_waits(nc, mybir)
    return nc


# ----------------------------------------------------------------------------
# host glue
# ----------------------------------------------------------------------------

def _prep_inputs(inputs, k):
    p = k // 2
    WIN = PS + 2 * p
    WELEM = WIN * WIN
    x = np.ascontiguousarray(np.asarray(inputs['x'], dtype=np.float32))
    xpad = np.pad(x[:, 0], ((0, 0), (p, p), (p, p)), mode='reflect')

    import ml_dtypes
    BFH = ml_dtypes.bfloat16

    wq = np.asarray(inputs['Wq'], np.float32)
    wk = np.asarray(inputs['Wk'], np.float32)
    wv = np.asarray(inputs['Wv'], np.float32)
    ident = np.eye(128, dtype=np.float32)

    TOT16 = 2 * WELEM + 6 * HD + 128 + 2 * NP_
    CB = 4 * HD + k

    sb = np.zeros((HD, 3 * HD + 3), np.float32)
    sb[:, 0:HD] = np.asarray(inputs['Wsq'], np.float32)
    sb[:, HD:2 * HD] = np.asarray(inputs['Wsk'], np.float32)
    sb[:, 2 * HD:3 * HD] = np.asarray(inputs['Wsv'], np.float32)
    sb[:, 3 * HD:3 * HD + 3] = np.asarray(inputs['Wp'], np.float32)

    bvec = np.zeros((HD, 5), np.float32)
    bvec[:, 0] = np.asarray(inputs['bq'], np.float32)
    bvec[:, 1] = np.asarray(inputs['bk'], np.float32)
    bvec[:, 2] = np.asarray(inputs['bsq'], np.float32)
    bvec[:, 3] = np.asarray(inputs['bsk'], np.float32)
    bvec[0:3, 4] = np.asarray(inputs['bp'], np.float32)

    offs = np.arange(-p, p + 1, dtype=np.float32)
    cf = np.zeros((128, 128 + CB), np.float32)
    cf[:, 0:128] = ident
    cf[:, 128:128 + HD] = np.asarray(inputs['bv'], np.float32).reshape(1, HD)
    cf[:, 128 + HD:128 + 2 * HD] = np.asarray(inputs['bsv'], np.float32).reshape(1, HD)
    cf[:, 128 + 2 * HD:128 + 3 * HD] = np.asarray(inputs['ln_g'], np.float32).reshape(1, HD)
    cf[:, 128 + 3 * HD:128 + 4 * HD] = np.asarray(inputs['ln_b'], np.float32).reshape(1, HD)
    cf[:, 128 + 4 * HD:128 + 4 * HD + k] = (offs ** 2).reshape(1, k)

    shared = {'cf32': cf, 'smallb': sb.astype(BFH), 'bvec': bvec}

    in_maps = []
    for c in range(NCORES):
        b, s = divmod(c, NCORES // B)
        slab = xpad[b, RPC * s: RPC * s + RPC + 2 * p, :]
        winv = np.lib.stride_tricks.sliding_window_view(slab, (WIN, WIN))[::PS, ::PS]
        xh = np.ascontiguousarray(winv.reshape(NBLK, WELEM), dtype=np.float32)
        xi = x[b, 0]
        pt = xi.reshape(H // PS, PS, W // PS, PS).transpose(1, 3, 0, 2).reshape(DIN, NP_)
        lo = NBLK * s
        perm = np.r_[lo:lo + NBLK, 0:lo, lo + NBLK:NP_]
        pt = np.ascontiguousarray(pt[:, perm], dtype=np.float32)
        big = np.zeros((128, TOT16), np.float32)
        big[:, 0:WELEM] = xh[0:128]
        big[:, WELEM:2 * WELEM] = xh[128:256]
        O_W = 2 * WELEM
        big[:, O_W + 0 * HD:O_W + 1 * HD] = wq[0:128]
        big[:, O_W + 1 * HD:O_W + 2 * HD] = wq[128:256]
        big[:, O_W + 2 * HD:O_W + 3 * HD] = wk[0:128]
        big[:, O_W + 3 * HD:O_W + 4 * HD] = wk[128:256]
        big[:, O_W + 4 * HD:O_W + 5 * HD] = wv[0:128]
        big[:, O_W + 5 * HD:O_W + 6 * HD] = wv[128:256]
        O_IDB = O_W + 6 * HD
        big[:, O_IDB:O_IDB + 128] = ident
        big[:, O_IDB + 128:O_IDB + 128 + NP_] = pt[0:128]
        big[:, O_IDB + 128 + NP_:O_IDB + 128 + 2 * NP_] = pt[128:256]
        m = dict(shared)
        m['big16'] = big.astype(BFH)
        in_maps.append(m)
    return in_maps


def _assemble(results):
    out = np.empty((B, C, H, W), dtype=np.float32)
    for c in range(NCORES):
        b, s = divmod(c, NCORES // B)
        r = results[c]['out']  # [NBLK, PIX]
        blk = r.reshape(BRC, WB, PS, PS).transpose(0, 2, 1, 3).reshape(RPC, W)
        out[b, 0, RPC * s: RPC * s + RPC, :] = blk
    return out


def kernel(**inputs):
    from concourse.bass_utils import run_bass_kernel_spmd
    k = _pick_k(inputs)
    nact = int(os.environ.get('AGBF_NACT', '4'))
    depri = int(os.environ.get('AGBF_DEPRI', '1'))
    nc = _build(k, nact, depri)
    in_maps = _prep_inputs(inputs, k)
    res = run_bass_kernel_spmd(nc, in_maps, core_ids=list(range(NCORES)),
                               trace=bool(int(os.environ.get('AGBF_TRACE', '0'))))
    out = _assemble(res.results)
    if os.environ.get('AGBF_TRACE', '0') != '0':
        kernel.last_exec_time_ns = res.exec_time_ns
    return out
